# revision 20
# baseline (speedup 1.0000x reference)
"""Trainium2 Bass kernel for 3D neighborhood attention (sparse_attention).

Problem: q,k [1,40,40,40,48] fp32, rpb [8,3,3,3]; out [1,24,40,40,40].
Per voxel x: logits[h,kk] = scale * <q[x,h,:], k[x+off_kk,h,:]> + rpb[h,kk]
(zero-padded k at boundaries, kk over 3x3x3 offsets), p = softmax over kk,
out[x,h,:] = sum_kk p[h,kk] * off_kk  (constant integer offsets as values).

Sharding: spatial-parallel over H (40 -> 8 slabs of 5). Tokens are mapped to
SBUF partitions in t-contiguous runs of 8 (fifth of a t-row), so the dl
(t-axis) shift of the 3x3x3 window is a free-dim AP offset into a 10-slot
halo. The host therefore only im2cols the 9 (di,dj) plane shifts ->
~9MB/core HBM instead of 27x (21.8MB); the DMA stream stays ahead of
compute. 8 tiles: 7x128 partitions + 1x104.

Engine split per tile (128 partitions x 8 tokens):
 - DVE: 3 muls (one per dl, reading the dl-shifted j-window of the k9 tile)
   producing fp16 products [d6][didj9, j8, dl3, h8]; then the factorized
   (di -> dj -> dl) fold tree over exp'd logits yielding the softmax
   denominator and three directional numerators, plus reciprocal + scale.
   All mul APs keep innermost contiguous runs that hold the DVE in fp16
   2x_1P packed mode.
 - TensorE: d-reduction as accumulating identity matmuls into PSUM
   (rpb seed + 6 d-blocks per 432-wide chunk, 4 chunks/tile).
 - ScalarE: exp straight from PSUM into fp16 SBUF.
 The DVE fold for tile i is emitted during tile i+1 (software pipelining);
 the output DMA is dispatched from the GpSimd DGE queue so it never
 head-of-line-blocks the exp activations or input DMAs; tiles 0-1 DMA +
 mul per d-half to shorten the ramp.
"""

import numpy as np

import concourse.bass as bass
import concourse.tile as tile
from concourse import bacc, mybir
from concourse.bass_utils import run_bass_kernel_spmd

F16 = np.float16

NH = 8
HD = 6
DIM = NH * HD
KS = 3
NT = KS**3  # 27
ND = KS * KS  # 9 (di,dj) plane shifts
SCALE = HD**-0.5
H = W = T = 40
N_CORES = 8
SLAB = H // N_CORES          # 5 rows of H per core
TOK = SLAB * W * T           # 8000 tokens per core
P = 128
TPP = 8                      # tokens per partition (fifth of a t-row)
JH = TPP + 2                 # j slots incl dl halo
NPART = TOK // TPP           # 1000 partitions of real data
TILES = 8                    # 7 full tiles + 1 tile of 104 partitions
PT = [P] * 7 + [NPART - 7 * P]   # partitions per tile
FKH = NT * NH                # 216 logits per token
LPP = TPP * FKH              # 1728 logits per partition
NCHUNK = 4
CHUNK = LPP // NCHUNK        # 432 logits (<=512 fp32, one PSUM bank)
KFREE = HD * ND * JH * NH    # 4320 k9 elems per partition
QFREE = HD * TPP * NH        # 384
DLB = LPP // KS              # 576 = one dl block (g, j, h)

_prog_cache = {}


def _build_program():
    fp16 = mybir.dt.float16
    fp32 = mybir.dt.float32
    nc = bacc.Bacc("TRN2", target_bir_lowering=False, debug=False,
                   num_devices=N_CORES)
    # free layouts (per partition):
    #   qs : (d6, j8, h8)             = 384
    #   k9 : (d6, didj9, jh10, h8)    = 4320
    #   rpb: (di3, dj3, j8, dl3, h8)  = 1728 (replicated over j)
    #   prod: (d6, didj9, j8, dl3, h8) per d-block 1728
    #   out: (o3, j8, h8)             = 192
    qs = nc.dram_tensor("qs", [TILES, P, QFREE], fp16,
                        kind="ExternalInput").ap()
    k9 = nc.dram_tensor("k9", [TILES, P, KFREE], fp16,
                        kind="ExternalInput").ap()
    rpbt = nc.dram_tensor("rpbt", [P, LPP], fp16, kind="ExternalInput").ap()
    ident_in = nc.dram_tensor("ident", [P, P], fp16,
                              kind="ExternalInput").ap()
    out = nc.dram_tensor("out", [TILES, P, 3 * TPP * NH], fp32,
                         kind="ExternalOutput").ap()

    J = TPP

    with tile.TileContext(nc) as tc:
        with (
            tc.tile_pool(name="consts", bufs=1) as cpool,
            tc.tile_pool(name="kin", bufs=4) as kpool,
            tc.tile_pool(name="qin", bufs=2) as qpool,
            tc.tile_pool(name="prod", bufs=3) as ppool,
            tc.tile_pool(name="psum", bufs=8, space="PSUM") as pspool,
            tc.tile_pool(name="expv", bufs=3) as epool,
            tc.tile_pool(name="l1", bufs=2) as l1pool,
            tc.tile_pool(name="l3", bufs=2) as l3pool,
            tc.tile_pool(name="tt", bufs=2) as ttpool,
            tc.tile_pool(name="small", bufs=8) as spool,
            tc.tile_pool(name="outp", bufs=2) as opool,
        ):
            rpb_rep = cpool.tile([P, LPP], fp16)
            nc.scalar.dma_start(rpb_rep[:], rpbt[:])
            ident = cpool.tile([P, P], fp16)
            nc.scalar.dma_start(ident[:], ident_in[:])

            state = {}

            def emit_front(ti):
                """DMA + QK mul + TensorE d-fold + exp for tile ti."""
                np_ = PT[ti]
                qt = qpool.tile([P, QFREE], fp16)
                # early tiles load k per-d-half so muls start while the DMA
                # stream is catching up, and put the first k half ahead of
                # q in the queue; later tiles use single transfers
                nparts = 2 if ti <= 1 else 1
                dpp = HD // nparts
                PSZ = dpp * ND * JH * NH
                ktp = kpool.tile([P, KFREE], fp16)
                nc.sync.dma_start(ktp[:np_, :PSZ], k9[ti, :np_, :PSZ])
                nc.sync.dma_start(qt[:np_], qs[ti, :np_])
                for pi in range(1, nparts):
                    nc.sync.dma_start(
                        ktp[:np_, pi * PSZ:(pi + 1) * PSZ],
                        k9[ti, :np_, pi * PSZ:(pi + 1) * PSZ])

                # seed the psum chunks with rpb first: these matmuls only
                # read consts, so TensorE runs them while the DVE muls work
                pcs = []
                for c in range(NCHUNK):
                    pc = pspool.tile([P, CHUNK], fp32)
                    nc.tensor.matmul(
                        pc[:np_], ident[:np_, :np_],
                        rpb_rep[:np_, c * CHUNK:(c + 1) * CHUNK],
                        start=True, stop=False)
                    pcs.append(pc)

                pt = ppool.tile([P, HD * LPP], fp16)
                kv = ktp[:np_].rearrange("p (d g jh h) -> p d g jh h",
                                         d=HD, g=ND, jh=JH)
                qv = (qt[:np_].rearrange("p (d j h) -> p d j h", d=HD, j=J)
                      .unsqueeze(2).broadcast_to([np_, HD, ND, J, NH]))
                pv = pt[:np_].rearrange(
                    "p (d g j dl h) -> p d g j dl h", d=HD, g=ND, j=J, dl=KS)
                # dst innermost runs of 8 (h) keep the DVE in fp16 2x_1P
                # packed mode. Early tiles mul per d-half so compute starts
                # as soon as the first half of the k9 DMA lands.
                for pi in range(nparts):
                    d0, d1 = pi * dpp, (pi + 1) * dpp
                    for dl in range(KS):
                        kop = kv[:, d0:d1, :, dl:dl + J]
                        nc.vector.tensor_mul(pv[:, d0:d1, :, :, dl],
                                             kop, qv[:, d0:d1])

                # logits into PSUM: per chunk, seed with rpb then accumulate
                # the 6 d-blocks via identity matmuls
                et = epool.tile([P, LPP], fp16)
                for c in range(NCHUNK):
                    pc = pcs[c]
                    for b in range(HD):
                        nc.tensor.matmul(
                            pc[:np_], ident[:np_, :np_],
                            pt[:np_, b * LPP + c * CHUNK:
                               b * LPP + (c + 1) * CHUNK],
                            start=False, stop=(b == HD - 1))
                    nc.scalar.activation(et[:np_, c * CHUNK:(c + 1) * CHUNK],
                                         pc[:np_],
                                         mybir.ActivationFunctionType.Exp)
                state[ti] = et

            def emit_back(ti):
                """DVE fold tree + out for tile ti (runs one tile late).

                e layout: (dl3, di3, dj3, j8, h8). The tree contracts di,
                then dj, then dl; numerators use slice differences."""
                np_ = PT[ti]
                et = state.pop(ti)
                R576 = KS * J * KS * NH          # (dj, j, dl, h) slice size
                R192 = J * KS * NH               # (j, dl, h) slice size
                # level 1 (contract di): a0 = sum_di E, a1 = E[di2]-E[di0]
                ev = et[:np_].rearrange("p (di r) -> p di r", di=KS)
                tt = ttpool.tile([P, R576], fp16)
                nc.vector.tensor_add(tt[:np_], ev[:, 0], ev[:, 1])
                l1t = l1pool.tile([P, 2 * R576], fp16)   # (s2, dj, j, dl, h)
                a0f = l1t[:np_, :R576]
                a1f = l1t[:np_, R576:]
                nc.vector.tensor_add(a0f, tt[:np_], ev[:, 2])
                nc.vector.tensor_sub(a1f, ev[:, 2], ev[:, 0])

                # level 2 (contract dj) for a0 and a1 together
                lv = l1t[:np_].rearrange("p (s dj r) -> p s dj r", s=2, dj=KS)
                ut = spool.tile([P, 2 * R192], fp16)
                uv = ut[:np_].rearrange("p (s r) -> p s r", s=2)
                nc.vector.tensor_add(uv, lv[:, :, 0], lv[:, :, 1])
                # l3in slots: s=0: B0=sum_dj a0, s=1: C1=sum_dj a1, s=2: B1
                l3in = l3pool.tile([P, 3 * R192], fp16)
                sall = l3in[:np_, :2 * R192].rearrange("p (s r) -> p s r",
                                                       s=2)
                nc.vector.tensor_add(sall, uv, lv[:, :, 2])
                a0v = l1t[:np_, :R576].rearrange("p (dj r) -> p dj r", dj=KS)
                b1f = l3in[:np_, 2 * R192:]
                nc.vector.tensor_sub(b1f, a0v[:, 2], a0v[:, 0])

                # level 3 (contract dl): zt slots = (s0, N_di, N_dj, N_dl)
                l3v = l3in[:np_].rearrange("p (s j dl h) -> p s j dl h", s=3,
                                           j=J, dl=KS)
                wt = spool.tile([P, 3 * J * NH], fp16)
                wv = wt[:np_].rearrange("p (s j h) -> p s j h", s=3, j=J)
                nc.vector.tensor_add(wv, l3v[:, :, :, 0], l3v[:, :, :, 1])
                zt = spool.tile([P, 4 * J * NH], fp32)
                zv = zt[:np_, :3 * J * NH].rearrange("p (s j h) -> p s j h",
                                                     s=3, j=J)
                nc.vector.tensor_add(zv, wv, l3v[:, :, :, 2])
                b0v = l3v[:, 0]  # [p, j, dl, h]
                ndl = zt[:np_, 3 * J * NH:].rearrange("p (j h) -> p j h", j=J)
                nc.vector.tensor_sub(ndl, b0v[:, :, 2], b0v[:, :, 0])

                # out[o, j, h] = N_o * (1/s0)
                rt = spool.tile([P, J * NH], fp32)
                nc.vector.reciprocal_approx_fast(rt[:np_], zt[:np_, :J * NH])
                ot = opool.tile([P, 3 * TPP * NH], fp32)
                r_b = (rt[:np_].rearrange("p (j h) -> p j h", j=J)
                       .unsqueeze(1).broadcast_to([np_, 3, J, NH]))
                nc.vector.tensor_mul(
                    ot[:np_].rearrange("p (o j h) -> p o j h", o=3, j=J),
                    zt[:np_, J * NH:].rearrange("p (o j h) -> p o j h", o=3,
                                                j=J),
                    r_b,
                )
                eng = nc.sync if ti == TILES - 1 else nc.gpsimd
                eng.dma_start(out[ti, :np_], ot[:np_])

            for ti in range(TILES):
                emit_front(ti)
                if ti >= 1:
                    emit_back(ti - 1)
            emit_back(TILES - 1)

    nc.compile()
    return nc


def _host_prep(q, k, rpb):
    q = np.asarray(q, dtype=np.float32)
    k = np.asarray(k, dtype=np.float32)
    rpb = np.asarray(rpb, dtype=np.float32)

    q0 = (q[0] * SCALE).astype(F16)                 # [40,40,40,48]
    kp = np.pad(k[0], ((1, 1), (1, 1), (1, 1), (0, 0)))  # [42,42,42,48]
    # 9 plane shifts: win9[g, x, y, tpad, c] = kp[x+di, y+dj, tpad, c]
    # with tpad covering t = -1..40 (12-slot windows are slices of this)
    win9 = np.empty((ND, H, W, T + 2, DIM), F16)
    for g in range(ND):
        di, dj = g // KS, g % KS
        win9[g] = kp[di:di + H, dj:dj + W, :, :]

    # rpb replicated over j: [di, dj, j8, dl, h]
    rpb_dh = rpb.transpose(1, 2, 3, 0)              # [di, dj, dl, h]
    rpb_rep = np.broadcast_to(rpb_dh[:, :, None],
                              (KS, KS, TPP, KS, NH)).reshape(LPP)
    rpb_t = np.broadcast_to(rpb_rep.astype(F16), (P, LPP)).copy()
    ident = np.eye(P, dtype=F16)

    in_maps = []
    for i in range(N_CORES):
        h0 = i * SLAB
        # partitions: token runs of 10 along t; rows = (x, y, tq)
        # q: [x, y, tq, 10, nh, hd] -> [part, d, j, h]
        qs_ = q0[h0:h0 + SLAB].reshape(SLAB * W, T // TPP, TPP, NH, HD)
        q_t = np.zeros((TILES * P, HD, TPP, NH), F16)
        q_t[:NPART] = qs_.reshape(NPART, TPP, NH, HD).transpose(0, 3, 1, 2)
        q_t = q_t.reshape(TILES, P, QFREE)

        # k9: per partition (x,y,tq): [d, didj, jh12, h]
        # win9 slab for this core's rows, with j-halo in t
        w9 = win9[:, h0:h0 + SLAB]                  # [9, 5, 40, 42, 48]
        # windows: partition (xy, tq) covers tpad slots tq*10 .. tq*10+11
        w9 = w9.reshape(ND, SLAB * W, T + 2, NH, HD)
        k_t = np.zeros((TILES * P, HD, ND, JH, NH), F16)
        kparts = np.empty((SLAB * W, T // TPP, ND, JH, NH, HD), F16)
        for tq in range(T // TPP):
            kparts[:, tq] = w9[:, :, tq * TPP:tq * TPP + JH].transpose(
                1, 0, 2, 3, 4)
        # [xy, tq, g, jh, h, d] -> [part, d, g, jh, h]
        k_t[:NPART] = kparts.reshape(NPART, ND, JH, NH, HD).transpose(
            0, 4, 1, 2, 3)
        k_t = k_t.reshape(TILES, P, KFREE)

        in_maps.append({"qs": q_t, "k9": k_t, "rpbt": rpb_t,
                        "ident": ident})
    return in_maps


def _assemble(results):
    slabs = []
    for i in range(N_CORES):
        o = results[i]["out"].reshape(TILES * P, 3, TPP, NH)[:NPART]
        o = o.transpose(0, 2, 1, 3).reshape(SLAB * W, T // TPP, TPP, 3, NH)
        o = o.reshape(SLAB, W, T, 3, NH)
        # channel order in reference: c = h*3 + o
        slabs.append(o.transpose(0, 1, 2, 4, 3).reshape(SLAB, W, T, 3 * NH))
    full = np.concatenate(slabs, axis=0)             # [40,40,40,24]
    return np.ascontiguousarray(full.transpose(3, 0, 1, 2))[None]


def _run(q, k, rpb, **spmd_kwargs):
    if "prog" not in _prog_cache:
        _prog_cache["prog"] = _build_program()
    nc = _prog_cache["prog"]
    in_maps = _host_prep(q, k, rpb)
    res = run_bass_kernel_spmd(nc, in_maps, list(range(N_CORES)),
                               **spmd_kwargs)
    return _assemble(res.results), res


def kernel(q, k, rpb):
    out, _ = _run(q, k, rpb)
    return out
